# revision 9
# baseline (speedup 1.0000x reference)
"""Trainium2 Bass kernel for a dense transformer encoder layer.

Model (see reference):
    kqv = x @ W_kqv ; split k,q,v ; multi-head attention (H=8, Hd=64)
    h   = gelu(attn_out @ W1 + b1) ; ffn = h @ W2 + b2
    out = LayerNorm(ffn)*gamma + beta + mean-pooled residual of x

Sharding: 8 cores, data-parallel over (batch, query-block).  Core c
handles batch n = c//4 and query-row block qb = c%4 (512 rows).  K/V are
computed per-core over the full 2048 keys of the core's batch.

Host->device traffic dominates the measured time for this problem (the
grader's timing window includes the input upload), so the input side is
minimized and consolidated:
  * the all-zero attn_mask (per the problem's input spec) is detected on
    host and never shipped (134MB saved); a masked variant is kept as a
    correctness fallback for nonzero masks;
  * all bf16 inputs (W_kqv, W1, W2, x^T) ship as ONE flat blob param;
  * x^T ships key-ROLLED per core so the core's own 512 query columns are
    columns 0:511 -- softmax/attn@v are reductions over keys, so a key
    permutation is harmless, and the query slice becomes a free SBUF view
    (no separate xtq tensor);
  * the residual is computed on-device from x^T (PE transpose) instead of
    shipping a row-major x copy;
  * biases/gamma/beta ship as one small f32 row param, partition-broadcast
    by DMA on device;
  * outputs are written (and downloaded) as bf16, upcast on host.

Layout strategy ("transposed attention"): all attention tensors keep the
head-dim / feature-dim on partitions so no on-chip transposes are needed:
    qT,kT : [Hd, rows]   from  W.T @ x.T
    sT    : [keys, qrows] = kT_tile.T @ qT
    exp(sT) with no max-subtraction (scores are O(1))
    outT  : v_aug.T @ exp(sT) accumulated over key tiles, where v_aug has
            a ones column => row 64 of the PSUM tile is the softmax
            denominator for each query.
    outT is exactly the lhsT layout the FFN matmuls need, so the whole
    network runs transpose-free.

All matmuls run in bf16 (fp32 PSUM accumulation).
"""

import numpy as np
import ml_dtypes

import concourse.bass as bass
import concourse.mybir as mybir
import concourse.tile as tile
from concourse import bacc

F32 = mybir.dt.float32
BF16 = mybir.dt.bfloat16
AF = mybir.ActivationFunctionType
ALU = mybir.AluOpType

N, L, D, H, HD, DFF, DOUT = 2, 2048, 512, 8, 64, 2048, 256
NCORES = 8
LQ = N * L // NCORES          # 512 query rows per core
KT = L // 128                 # 16 key tiles
DCH = D // 128                # 4 contraction chunks of D
FBLK = DFF // 128             # 16 dff blocks
QTL = LQ // 128               # 4 query sub-tiles (output rows)
GRP = NCORES // N             # 4 cores per batch
LN_EPS = 1e-5

GELU_FUNC = AF.Gelu

# flat bf16 blob layout (element offsets)
OFF_WKQV = 0
N_WKQV = DCH * 128 * 3 * D            # [DCH,128,3D]
OFF_W1 = OFF_WKQV + N_WKQV
N_W1 = DCH * 128 * DFF                # [DCH,128,DFF]
OFF_W2 = OFF_W1 + N_W1
N_W2 = FBLK * 128 * DOUT              # [FBLK,128,DOUT]
OFF_XT = OFF_W2 + N_W2
N_XT = DCH * 128 * L                  # [DCH,128,L] (key-rolled per core)
OFF_SM = OFF_XT + N_XT                # b1c [128,FBLK] ++ b2/gamma/beta (bf16)
NSMALL = 128 * FBLK + 3 * DOUT
NBIG = OFF_SM + NSMALL


def _emit(nc, reps=1, masked=False):
    """Emit the whole per-core program under a TileContext."""
    dp = nc.declare_dram_parameter
    big = dp("big", [1, NBIG], BF16, isOutput=False)
    if masked:
        # key axis rolled by q0 to match the rolled x^T
        maskT = dp("maskT", [H, KT // 4, 128, 4, LQ], BF16, isOutput=False)
    out = dp("out", [QTL, 128, DOUT], BF16, isOutput=True)

    bigf = big[0]
    smallf = bigf[OFF_SM:OFF_SM + NSMALL]

    def bcast_row(flat_ap):
        return bass.AP(tensor=flat_ap.tensor, offset=flat_ap.offset,
                       ap=[[0, 128]] + [list(a) for a in flat_ap.ap])

    with tile.TileContext(nc) as tc:
      for _rep in range(reps):
        with (
            tc.tile_pool(name="const", bufs=1) as const,
            tc.tile_pool(name="mask", bufs=3) as maskp,
            tc.tile_pool(name="sexp", bufs=2) as sexpp,
            tc.tile_pool(name="norm", bufs=2) as normp,
            tc.tile_pool(name="ps_s", bufs=4, space="PSUM") as ps_s,
            tc.tile_pool(name="ps_o", bufs=2, space="PSUM") as ps_o,
        ):
            # ---------------- constant / input loads ----------------
            xt_sb = const.tile([128, DCH, L], BF16)
            wkqv_sb = const.tile([128, DCH, 3 * D], BF16)
            w1_sb = const.tile([128, DCH, DFF], BF16)
            w2_sb = const.tile([128, FBLK, DOUT], BF16)
            b1_sb = const.tile([128, FBLK], F32)
            b2b_sb = const.tile([128, DOUT], F32)
            gamma_sb = const.tile([128, DOUT], F32)
            beta_sb = const.tile([128, DOUT], F32)
            eps_sb = const.tile([128, 1], F32)
            ident_sb = const.tile([128, 128], BF16, name="ident")

            # chunked loads so compute can start on the first chunk
            for ch in range(DCH):
                nc.gpsimd.dma_start(
                    xt_sb[:, ch, :],
                    bigf[OFF_XT + ch * 128 * L:OFF_XT + (ch + 1) * 128 * L]
                    .rearrange("(p l) -> p l", p=128))
                nc.gpsimd.dma_start(
                    wkqv_sb[:, ch, :],
                    bigf[ch * 128 * 3 * D:(ch + 1) * 128 * 3 * D]
                    .rearrange("(p f) -> p f", p=128))
            for ch in range(DCH):
                nc.gpsimd.dma_start(
                    w1_sb[:, ch, :],
                    bigf[OFF_W1 + ch * 128 * DFF:OFF_W1 + (ch + 1) * 128 * DFF]
                    .rearrange("(p f) -> p f", p=128))
            nc.gpsimd.dma_start(
                w2_sb,
                bigf[OFF_W2:OFF_W2 + N_W2]
                .rearrange("(f p d) -> p f d", f=FBLK, p=128))
            b1h_sb = const.tile([128, FBLK], BF16, name="b1h")
            bgbh_sb = const.tile([128, 3, DOUT], BF16, name="bgbh")
            nc.gpsimd.dma_start(
                b1h_sb, smallf[0:128 * FBLK].rearrange("(p f) -> p f", p=128))
            ob0 = 128 * FBLK
            nc.gpsimd.dma_start(bgbh_sb[:, 0, :],
                                bcast_row(smallf[ob0:ob0 + DOUT]))
            nc.gpsimd.dma_start(bgbh_sb[:, 1, :],
                                bcast_row(smallf[ob0 + DOUT:ob0 + 2 * DOUT]))
            nc.gpsimd.dma_start(bgbh_sb[:, 2, :],
                                bcast_row(smallf[ob0 + 2 * DOUT:ob0 + 3 * DOUT]))
            nc.vector.tensor_copy(b1_sb, b1h_sb)
            nc.vector.tensor_copy(b2b_sb, bgbh_sb[:, 0, :])
            nc.vector.tensor_copy(gamma_sb, bgbh_sb[:, 1, :])
            nc.vector.tensor_copy(beta_sb, bgbh_sb[:, 2, :])
            nc.vector.memset(eps_sb, LN_EPS)
            from concourse.masks import make_identity
            make_identity(nc, ident_sb)

            xtq_sb = xt_sb[:, :, 0:LQ]   # own query columns (rolled to front)

            # resid^T = xtq[chunks 0:2] + xtq[chunks 2:4]  (x0.5 applied later)
            rsb = const.tile([128, 2, LQ], BF16, name="rsb")
            nc.vector.tensor_add(rsb, xtq_sb[:, 0:2, :], xtq_sb[:, 2:4, :])

            kT_sb = const.tile([128, DCH, L], BF16, name="kT")
            qT_sb = const.tile([128, DCH, LQ], BF16, name="qT")
            attn_sb = const.tile([128, DCH, LQ], BF16, name="attn")
            v_sb = []

            def emit_v(ps_pool):
                for kt in range(KT):
                    ps = ps_pool.tile([128, D], F32, name="ps_qkv")
                    for ch in range(DCH):
                        nc.tensor.matmul(ps, xt_sb[:, ch, kt * 128:(kt + 1) * 128],
                                         wkqv_sb[:, ch, 2 * D:3 * D],
                                         start=(ch == 0), stop=(ch == DCH - 1))
                    vt = const.tile([128, H, HD + 1], BF16, name=f"v_{kt}")
                    nc.scalar.activation(vt[:, :, 0:HD],
                                         ps.rearrange("p (h d) -> p h d", h=H),
                                         AF.Copy)
                    nc.vector.memset(vt[:, :, HD:HD + 1], 1.0)
                    v_sb.append(vt)

            def emit_kT(ps_pool, ob):
                for lb in range(L // 512):
                    ps = ps_pool.tile([128, 512], F32, name="ps_qkv")
                    for ch in range(DCH):
                        nc.tensor.matmul(
                            ps, wkqv_sb[:, ch, ob * 128:(ob + 1) * 128],
                            xt_sb[:, ch, lb * 512:(lb + 1) * 512],
                            start=(ch == 0), stop=(ch == DCH - 1))
                    nc.vector.tensor_copy(kT_sb[:, ob, lb * 512:(lb + 1) * 512], ps)

            def emit_qT(ps_pool, ob):
                ps = ps_pool.tile([128, LQ], F32, name="ps_qkv")
                for ch in range(DCH):
                    nc.tensor.matmul(ps,
                                     wkqv_sb[:, ch, D + ob * 128:D + (ob + 1) * 128],
                                     xtq_sb[:, ch, :],
                                     start=(ch == 0), stop=(ch == DCH - 1))
                nc.scalar.activation(qT_sb[:, ob, :], ps, AF.Copy,
                                     scale=1.0 / np.sqrt(HD))

            def emit_norm(h, o_ps):
                ob, po = h // 2, (h % 2) * 64
                osb = normp.tile([128, LQ], F32, name="osb")
                nc.vector.tensor_copy(osb[0:HD + 1, :], o_ps[0:HD + 1, :])
                nc.vector.reciprocal(osb[HD:HD + 1, :], osb[HD:HD + 1, :])
                recipB = normp.tile([128, LQ], F32, name="recipB")
                rsrc = osb[HD:HD + 1, :]
                rap = list(rsrc.ap)
                nc.gpsimd.dma_start(
                    recipB[po:po + 64, :],
                    bass.AP(tensor=rsrc.tensor, offset=rsrc.offset,
                            ap=[list(rap[0]), [0, 64]] + [list(a) for a in rap[1:]]))
                if po == 0:
                    nc.vector.tensor_mul(attn_sb[0:64, ob, :],
                                         osb[0:64, :], recipB[0:64, :])
                else:
                    stage = normp.tile([128, LQ], F32, name="stage")
                    nc.gpsimd.dma_start(stage[64:128, :], osb[0:64, :])
                    nc.vector.tensor_mul(attn_sb[64:128, ob, :],
                                         stage[64:128, :], recipB[64:128, :])

            def emit_head(h):
                """Stage 1: stream scores->(mask)->exp for all 16 key tiles
                into SBUF.  Stage 2: pure-PE burst of the 16 accumulating
                attn@v matmuls.  Heads pipeline on PE vs DVE/ACT."""
                ob, po = h // 2, (h % 2) * 64
                e_tiles = []
                for g in range(KT // 4):
                    if masked:
                        m_sb = maskp.tile([128, 4, LQ], BF16, name="m")
                        nc.sync.dma_start(m_sb, maskT[h, g])
                    for k in range(4):
                        kt = g * 4 + k
                        s_ps = ps_s.tile([128, LQ], F32, name="s_ps")
                        nc.tensor.matmul(s_ps,
                                         kT_sb[po:po + 64, ob,
                                               kt * 128:(kt + 1) * 128],
                                         qT_sb[po:po + 64, ob, :],
                                         start=True, stop=True)
                        if masked:
                            nc.vector.tensor_add(s_ps, s_ps, m_sb[:, k, :])
                        e_sb = sexpp.tile([128, LQ], BF16, name=f"e_{kt}")
                        nc.scalar.activation(e_sb, s_ps, AF.Exp)
                        e_tiles.append(e_sb)
                o_ps = ps_o.tile([128, LQ], F32, name="o_ps")
                for kt in range(KT):
                    nc.tensor.matmul(o_ps[:HD + 1, :], v_sb[kt][:, h, :],
                                     e_tiles[kt], start=(kt == 0),
                                     stop=(kt == KT - 1))
                emit_norm(h, o_ps)

            # qkv psum pool scoped: closes before the FFN pools open so the
            # FFN psum banks only wait on (early) qkv reads, not attention
            with tc.tile_pool(name="ps_qkv", bufs=2, space="PSUM") as ps_qkv:
                emit_v(ps_qkv)
                for ob in range(DCH):
                    emit_kT(ps_qkv, ob)
                    emit_qT(ps_qkv, ob)

            # ---------------- attention + FFN (overlapping pools) ----------
            with (
                tc.tile_pool(name="hbuf", bufs=1) as hpool,
                tc.tile_pool(name="ffn", bufs=2) as ffnp,
                tc.tile_pool(name="ps_f", bufs=2, space="PSUM") as ps_f1,
            ):
                for h in range(H):
                    emit_head(h)

                h_sb = []
                for fb in range(FBLK):
                    ps = ps_f1.tile([128, LQ], F32, name="ps_h")
                    for ch in range(DCH):
                        nc.tensor.matmul(ps, w1_sb[:, ch, fb * 128:(fb + 1) * 128],
                                         attn_sb[:, ch, :],
                                         start=(ch == 0), stop=(ch == DCH - 1))
                    ht = hpool.tile([128, LQ], BF16, name=f"h_{fb}")
                    nc.scalar.activation(ht, ps, GELU_FUNC, bias=b1_sb[:, fb:fb + 1])
                    h_sb.append(ht)

                for qt in range(QTL):
                    ps2 = ps_f1.tile([128, DOUT], F32, name="ps_h")
                    for fb in range(FBLK):
                        nc.tensor.matmul(ps2, h_sb[fb][:, qt * 128:(qt + 1) * 128],
                                         w2_sb[:, fb, :],
                                         start=(fb == 0), stop=(fb == FBLK - 1))
                    nc.vector.tensor_add(ps2, ps2, b2b_sb)
                    stats = ffnp.tile([128, 6], F32, name="stats")
                    nc.vector.bn_stats(stats, ps2)
                    mv = ffnp.tile([128, 2], F32, name="mv")
                    nc.vector.bn_aggr(mv, stats)
                    sd = ffnp.tile([128, 1], F32, name="sd")
                    nc.scalar.activation(sd, mv[:, 1:2], AF.Sqrt, bias=eps_sb)
                    rstd = ffnp.tile([128, 1], F32, name="rstd")
                    nc.vector.reciprocal(rstd, sd)
                    t_sb = ffnp.tile([128, DOUT], F32, name="t")
                    nc.vector.tensor_scalar(t_sb, ps2, mv[:, 0:1], rstd,
                                            op0=ALU.subtract, op1=ALU.mult)
                    nc.vector.tensor_mul(t_sb, t_sb, gamma_sb)
                    # residual: transpose rsb blocks on PE -> [q, 256]
                    ps_r = ps_f1.tile([128, DOUT], F32, name="ps_h")
                    for j in range(2):
                        nc.tensor.matmul(ps_r[:, j * 128:(j + 1) * 128],
                                         rsb[:, j, qt * 128:(qt + 1) * 128],
                                         ident_sb, start=True, stop=True)
                    r2 = ffnp.tile([128, DOUT], F32, name="r2")
                    nc.vector.scalar_tensor_tensor(r2, ps_r, 0.5, beta_sb,
                                                   op0=ALU.mult, op1=ALU.add)
                    o_sb = ffnp.tile([128, DOUT], BF16, name="o_sb")
                    nc.vector.tensor_add(o_sb, t_sb, r2)
                    nc.sync.dma_start(out[qt], o_sb)
    return nc


_NC = {}


def _get_nc(reps=1, masked=False):
    key = (reps, masked)
    if key not in _NC:
        nc = bacc.Bacc(enable_partition_id=False)
        _emit(nc, reps, masked)
        nc.compile()
        _NC[key] = nc
    return _NC[key]


def _stage_inputs(x, attn_mask, W_kqv, W1, b1, W2, b2, gamma, beta, masked=False):
    """Build the 8 per-core input maps (host-side layout/dtype staging)."""
    bf = ml_dtypes.bfloat16
    x = np.asarray(x, np.float32)
    wpart = np.empty(OFF_XT, bf)
    wpart[OFF_WKQV:OFF_WKQV + N_WKQV] = \
        np.asarray(W_kqv, np.float32).astype(bf).ravel()
    wpart[OFF_W1:OFF_W1 + N_W1] = np.asarray(W1, np.float32).astype(bf).ravel()
    wpart[OFF_W2:OFF_W2 + N_W2] = np.asarray(W2, np.float32).astype(bf).ravel()
    small = np.empty(NSMALL, bf)
    small[:128 * FBLK] = np.ascontiguousarray(
        np.asarray(b1, np.float32).reshape(FBLK, 128).T).astype(bf).ravel()
    ob0 = 128 * FBLK
    small[ob0:ob0 + DOUT] = np.asarray(b2, np.float32).astype(bf)
    small[ob0 + DOUT:ob0 + 2 * DOUT] = np.asarray(gamma, np.float32).astype(bf)
    small[ob0 + 2 * DOUT:] = np.asarray(beta, np.float32).astype(bf)
    xT = [np.ascontiguousarray(x[n].T).astype(bf) for n in range(N)]  # [D, L]
    in_maps = []
    for c in range(NCORES):
        n, qb = divmod(c, GRP)
        q0 = qb * LQ
        big = np.empty((1, NBIG), bf)
        big[0, :OFF_XT] = wpart
        big[0, OFF_XT:OFF_SM] = np.roll(xT[n], -q0, axis=1).ravel()
        big[0, OFF_SM:] = small
        m = {"big": big}
        if masked:
            am = np.asarray(attn_mask, np.float32)
            # roll the key axis to match the rolled x^T keys
            mt = np.roll(am[n, :, q0:q0 + LQ, :], -q0, axis=2)
            mt = np.ascontiguousarray(mt.transpose(0, 2, 1))  # [H, L, LQ]
            mt = mt.reshape(H, KT // 4, 4, 128, LQ).transpose(0, 1, 3, 2, 4)
            m["maskT"] = np.ascontiguousarray(mt).astype(bf)
        in_maps.append(m)
    return in_maps


def kernel(x, attn_mask, W_kqv, W1, b1, W2, b2, gamma, beta, num_heads,
           _return_results=False, **_ignored):
    assert int(num_heads) == H
    from concourse.bass_utils import run_bass_kernel_spmd

    masked = bool(np.any(np.asarray(attn_mask)))
    nc = _get_nc(1, masked)
    in_maps = _stage_inputs(x, attn_mask, W_kqv, W1, b1, W2, b2, gamma, beta,
                            masked)
    res = run_bass_kernel_spmd(nc, in_maps, core_ids=list(range(NCORES)))
    full = np.empty((N, L, DOUT), np.float32)
    for c in range(NCORES):
        n, qb = divmod(c, GRP)
        q0 = qb * LQ
        full[n, q0:q0 + LQ, :] = \
            res.results[c]["out"].reshape(LQ, DOUT).astype(np.float32)
    if _return_results:
        return full, res
    return full
